# revision 72
# baseline (speedup 1.0000x reference)
"""Child-Sum TreeLSTM over a perfect binary tree (N=65535, depth 15) on 8 trn2 cores.

Sharding: each core owns one depth-3 subtree (levels 15..3 are fully local:
children of node range [a,b) are [2a+1,2b+1), so per-core level slices are
contiguous and child gathers are stride-2 local reads).  The 8 subtree roots
are AllGathered (16 KB) and the top 7 nodes are computed redundantly on every
core; the host takes them from core 0.

On-chip layout is feature-major ([feature-chunk=128 partitions, nodes free]);
the host pre-transposes the inputs so the device never transposes anything.
Biases are folded into the x-side matmul via an appended ones-row; K is
padded 301 -> 384 because matmuls with <128-partition weights run at half
the sustained PE rate (no FWL: hw-measured 427 vs 216 ns per 512-col MM).
Matmuls run in bf16 (fp32 PSUM accumulation); cell state is fp32.

Big levels run in 256-column blocks; every psum tile is exactly ONE 2KB bank
([2 banks-halves, 256] f32 holding an m-chunk pair), so a block holds 4-5 of
the 8 banks and ~2 blocks pipeline through PSUM.  The gate-chain tail (tanh,
h-mul, child-h sums, fpre enqueue) of each block is deferred by TWO blocks so
consecutive chains pipeline instead of serializing through the in-order
Scalar queue.  Child-h sums (hs) for the next level are produced on the DVE
as soon as each source block pair's h lands, into ping-pong hsbuf buffers,
so the parent's h-side matmuls never wait.  The f-gate h-side matmuls (fpre)
are deferred thunks flushed after the NEXT block's dense matmuls.  x arrives
via [128, 1024] supertiles prefetched 2 ahead on the sync queue; outputs
leave per level on the gpsimd queue (pure DMA-issue, so its waits block no
compute).  Two junk warm-up AllGathers run before the real root gather: the
first absorbs ncfw cold start, the second (input-gated on level-11 data)
fires ~15us before the root and realigns the cores, cutting arrival skew.
The serial small-level tail (S<=64) batches its x-side into persistent tiles
(plus a column-duplicated copy so per-child f-gates are single interleaved
passes) and reads PSUM directly from the activations.
"""

import sys

sys.path.insert(0, "/opt/trn_rl_repo")

import numpy as np
import ml_dtypes

IN_DIM = 300
MEM = 256
DEPTH = 15
N_NODES = 2 ** (DEPTH + 1) - 1  # 65535
NCORES = 8
SUB_DEPTH = 3  # shard at depth 3 -> 8 subtrees
NB = 256  # node block size: [2, 256] f32 = one 2KB psum bank per m-pair tile,
# so a block holds 4-5 of 8 banks and ~2 blocks pipeline through PSUM
SMALL_S = 64  # levels with S <= this use the batched x pass (3*S <= NB)
XSUP = 1024  # x prefetch supertile width (columns)
# K padded 301 -> 384: matmuls with <128-partition weights run at half the
# sustained PE rate (no FWL; hw-measured 427 vs 216 ns per 512-col MM), so
# the partial 45-row chunk is padded with zero weight rows to keep the whole
# stream warm.
K_PAD = 384

LEVELS = list(range(DEPTH, SUB_DEPTH - 1, -1))  # 15..3
S_OF = {d: 2 ** (d - SUB_DEPTH) for d in LEVELS}  # 4096..1
OFF_OF = {}
_off = 0
for _d in LEVELS:
    OFF_OF[_d] = _off
    _off += S_OF[_d]
N_LOCAL = _off  # 8191
TOP_COL0 = N_LOCAL  # columns 8191..8197 hold x of global nodes 0..6
N_COLS = N_LOCAL + 7  # 8198
_batch_levels = [d for d in LEVELS if S_OF[d] <= SMALL_S]
XSB_COL0 = OFF_OF[_batch_levels[0]]  # first column served by the batched x pass
XSB_N = N_COLS - XSB_COL0  # 134

_CACHE = {}


def _ceil_div(a, b):
    return -(-a // b)


def _build_program():
    import concourse.mybir as mybir
    import concourse.bacc as bacc
    from concourse import tile

    f32 = mybir.dt.float32
    bf16 = mybir.dt.bfloat16
    f8 = mybir.dt.float8e4
    DR = mybir.MatmulPerfMode.DoubleRow
    SIG = mybir.ActivationFunctionType.Sigmoid
    TANH = mybir.ActivationFunctionType.Tanh

    nc = bacc.Bacc("TRN2", target_bir_lowering=False, debug=False, num_devices=NCORES)

    # NOTE: fp8 DoubleRow for the x-side was tried and reverted: the HAM
    # power-throttles the PE to 4/8 clock while fp8 runs (min matmul dur
    # 426ns = 2x the bf16 floor), erasing the throughput gain for the whole
    # stream.
    xt = nc.dram_tensor("xt", [K_PAD, N_COLS], bf16, kind="ExternalInput")
    xdup = nc.dram_tensor("xdup", [K_PAD, 2 * XSB_N], bf16, kind="ExternalInput")
    wx = nc.dram_tensor("wx", [K_PAD, 4 * MEM], bf16, kind="ExternalInput")
    wh = nc.dram_tensor("wh", [MEM, 3 * MEM], bf16, kind="ExternalInput")
    wf = nc.dram_tensor("wf", [MEM, MEM], bf16, kind="ExternalInput")
    out = nc.dram_tensor("out", [2, 128, N_COLS], f32, kind="ExternalOutput")
    # h written in its native bf16 (the host upcasts): 25% less output DMA
    outh = nc.dram_tensor("outh", [2, 128, N_COLS], bf16, kind="ExternalOutput")

    KCH = [(0, 128), (128, 128), (256, 128)]  # k chunks of K_PAD=384

    with tile.TileContext(nc) as tc:
        with (
            tc.tile_pool(name="const", bufs=1) as cpool,
            tc.tile_pool(name="perst", bufs=1) as ppool,
            tc.tile_pool(name="xp", bufs=4) as xpool,
            tc.tile_pool(name="wk", bufs=2) as wk,
            tc.tile_pool(name="ps", bufs=8, space="PSUM") as psp,
            tc.tile_pool(name="dram", bufs=1, space="DRAM") as dram,
        ):
            # ---- PE warm-up: dense junk matmuls with no input deps run
            # during the initial DMA window so the HAM un-throttles the PE
            # clock (4/8 -> 8/8) before the real matmul stream begins
            jw = wk.tile([128, 128], bf16, tag="jw", name="jw", bufs=1)
            jx = wk.tile([128, NB], bf16, tag="jx", name="jx", bufs=1)
            nc.vector.memset(jw[:], 0.0)
            nc.vector.memset(jx[:], 0.0)
            pw = [
                psp.tile([128, 2, NB], f32, tag="ps", name=f"pw{j}") for j in range(2)
            ]
            # enough reps to bridge the whole startup DMA window (~13us) so
            # the PE stays warm until the first supertile lands
            for i in range(96):
                nc.tensor.matmul(
                    pw[(i // 2) % 2][:, i % 2, :],
                    jw[:],
                    jx[:],
                    start=True,
                    stop=True,
                )
            # consume the warm-up results so they are not dead-code-eliminated
            # (gin is fully overwritten by the real root DMAs later)
            jo = wk.tile([128, 2], f32, tag="jo", name="jo", bufs=1)
            nc.vector.tensor_copy(jo[:], pw[0][:, 0, 0:2])
            nc.vector.tensor_copy(jo[:], pw[1][:, 1, 0:2])

            # ---- load weights ----
            wx_sb = []
            for i, (k0, kn) in enumerate(KCH):
                t = cpool.tile([kn, 4 * MEM], bf16, tag=f"wx{i}", name=f"wx{i}")
                # quarter-column pieces: short per-engine chains at startup
                for q in range(4):
                    c0, c1 = q * MEM, (q + 1) * MEM
                    nc.sync.dma_start(t[:, c0:c1], wx[k0 : k0 + kn, c0:c1])
                wx_sb.append(t)

            # ---- x supertile prefetcher: [128, 1024]-col loads keep the
            # sync-queue issue count low and prefetch 2 supertiles ahead.
            # The first supertile is issued right after the wx loads so the
            # first leaf matmuls aren't starved behind wh/wf/xs loads. ----
            BIG_COLS = OFF_OF[_batch_levels[0]]  # big-level x range [0, BIG_COLS)
            N_SUP = _ceil_div(BIG_COLS, XSUP)
            sup_tiles = [None] * N_SUP
            sup_state = {"next": 0}

            def _load_sup(k):
                c0 = k * XSUP
                cn = min(XSUP, BIG_COLS - c0)
                ts_ = []
                for i, (k0, kn) in enumerate(KCH):
                    t = xpool.tile([kn, XSUP], bf16, tag=f"xk{i}", name=f"xs{k}_{i}")
                    # two half-column DMAs: twice the engines per supertile
                    h = cn // 2
                    nc.sync.dma_start(t[:, :h], xt[k0 : k0 + kn, c0 : c0 + h])
                    nc.sync.dma_start(
                        t[:, h:cn], xt[k0 : k0 + kn, c0 + h : c0 + cn]
                    )
                    ts_.append(t)
                sup_tiles[k] = ts_

            def get_x(col0):
                k = col0 // XSUP
                want = min(k + 3, N_SUP - 1)
                while sup_state["next"] <= want:
                    _load_sup(sup_state["next"])
                    sup_state["next"] += 1
                return sup_tiles[k], col0 - k * XSUP

            # first block's 256 columns load in small dedicated pieces ahead
            # of everything else so the first real matmuls start early
            sup0 = []
            for i, (k0, kn) in enumerate(KCH):
                t = xpool.tile([kn, XSUP], bf16, tag=f"xk{i}", name=f"xs0_{i}")
                nc.sync.dma_start(t[:, :NB], xt[k0 : k0 + kn, 0:NB])
                sup0.append(t)
            for i, (k0, kn) in enumerate(KCH):
                t = sup0[i]
                nc.sync.dma_start(t[:, NB:640], xt[k0 : k0 + kn, NB:640])
                nc.sync.dma_start(t[:, 640:XSUP], xt[k0 : k0 + kn, 640:XSUP])
            sup_tiles[0] = sup0
            sup_state["next"] = 1

            wh_sb = []
            for i in range(2):
                t = cpool.tile([128, 3 * MEM], bf16, tag=f"wh{i}", name=f"wh{i}")
                nc.sync.dma_start(t[:], wh[i * 128 : (i + 1) * 128, :])
                wh_sb.append(t)
            wf_sb = []
            for i in range(2):
                t = cpool.tile([128, MEM], bf16, tag=f"wf{i}", name=f"wf{i}")
                nc.sync.dma_start(t[:], wf[i * 128 : (i + 1) * 128, :])
                wf_sb.append(t)

            # ---- warm-up collective: a junk 1KB AllGather issued up front
            # keeps ncfw/the CC stream warm so the real root gather at the
            # end does not pay the cold-start trigger latency.
            gin_w = dram.tile([32, 1], f32)
            nc.sync.dma_start(gin_w[:, :], jo[0:32, 0:1])
            gout_w = dram.tile([32 * NCORES, 1], f32)
            nc.gpsimd.collective_compute(
                "AllGather",
                mybir.AluOpType.bypass,
                replica_groups=[list(range(NCORES))],
                ins=[gin_w.opt()],
                outs=[gout_w.opt()],
            )

            # ---- persistent level buffers (A = odd levels, B = even) ----
            hbuf = {
                1: ppool.tile([128, 2, 4096], bf16, tag="hA", name="hA"),
                0: ppool.tile([128, 2, 2048], bf16, tag="hB", name="hB"),
            }
            cbuf = {
                1: ppool.tile([128, 2, 4096], f32, tag="cA", name="cA"),
                0: ppool.tile([128, 2, 2048], f32, tag="cB", name="cB"),
            }
            fbuf = {
                1: ppool.tile([128, 2, 4096], bf16, tag="fA", name="fA"),
                0: ppool.tile([128, 2, 2048], bf16, tag="fB", name="fB"),
            }

            # ---- persistent x tiles for the tiny levels + top (134 cols),
            # plus a column-duplicated copy for the per-child f-gate pass.
            # Loads are EMITTED mid-kernel (at level 13) so the startup DMA
            # burst doesn't delay the leaf-phase supertiles. ----
            xs_small = []
            xs_dup = []
            for i, (k0, kn) in enumerate(KCH):
                xs_small.append(
                    cpool.tile([kn, XSB_N], bf16, tag=f"xs{i}", name=f"xs{i}")
                )
                xs_dup.append(
                    cpool.tile([kn, 2 * XSB_N], bf16, tag=f"xd{i}", name=f"xd{i}")
                )

            def load_xs_small():
                for i, (k0, kn) in enumerate(KCH):
                    nc.sync.dma_start(
                        xs_small[i][:], xt[k0 : k0 + kn, XSB_COL0:N_COLS]
                    )
                    nc.sync.dma_start(xs_dup[i][:], xdup[k0 : k0 + kn, :])

            def do_level(
                S,
                x_col0,
                out_col0,
                hs_src,  # bf16 AP [128, 2, S]: precomputed child-h sums, or None (leaf)
                c_child,  # f32 AP [128, 2, 2S] or None
                f_child,  # bf16 AP [128, 2, 2S] (fpre of children) or None
                h_dest,  # bf16 AP [128, 2, >=S]
                c_dest,  # f32 AP [128, 2, >=S]
                fpre_out,  # bf16 AP [128, 2, >=S] or None
                hs_sink=None,  # bf16 AP [128, 2, S//2]: this level's child-h
                # sums for the parent, produced as soon as each source block
                # pair's h lands (keeps the parent's h-side matmuls unblocked)
                pending=None,  # deferred fpre-matmul thunks (cross-level)
                tails=None,  # deferred gate-chain tails (cross-level)
            ):
                leaf = c_child is None
                sp = min(NB, S // 2) if hs_sink is not None else 0
                hs_done = [0]
                # the child level's last tails still sit in `tails`; they
                # produce the final hs chunk and fpre blocks this level's
                # matmuls read, so emit them before the block loop
                for th in tails:
                    th()
                tails.clear()
                for b in range(_ceil_div(S, NB)):
                    col = b * NB
                    s = min(NB, S - col)
                    n_m = 6 if leaf else 8
                    xts, xo = get_x(x_col0 + col)
                    pt = [
                        psp.tile([128, 2, NB], f32, tag="ps", name=f"pt{j}")
                        for j in range(n_m // 2)
                    ]
                    # chunk-major m-pairs: each pt[j] is ONE psum bank holding
                    # m=2j and m=2j+1; exactly one start (first matmul into
                    # the bank) and one stop (last matmul into the bank).
                    # Single-block levels emit ALL x-side matmuls first: the
                    # in-order tensor queue then runs them during the child
                    # level's still-draining gate chain instead of stalling
                    # at the first h-side matmul (which waits on hs).
                    xfirst = S <= NB and not leaf
                    for m in range(n_m):
                        msl = slice(m * 128, (m + 1) * 128)
                        lastx = m % 2 == 1 and (leaf or m == 7)
                        for ki in range(3):
                            nc.tensor.matmul(
                                pt[m // 2][:, m % 2, :s],
                                wx_sb[ki][:, msl],
                                xts[ki][:, xo : xo + s],
                                start=(ki == 0 and m % 2 == 0),
                                stop=(ki == 2 and lastx),
                            )
                        if not leaf and not xfirst and m < 6:
                            for hc in range(2):
                                nc.tensor.matmul(
                                    pt[m // 2][:, m % 2, :s],
                                    wh_sb[hc][:, msl],
                                    hs_src[:, hc, col : col + s],
                                    start=False,
                                    stop=(hc == 1 and m % 2 == 1),
                                )
                    if xfirst:
                        for m in range(6):
                            msl = slice(m * 128, (m + 1) * 128)
                            for hc in range(2):
                                nc.tensor.matmul(
                                    pt[m // 2][:, m % 2, :s],
                                    wh_sb[hc][:, msl],
                                    hs_src[:, hc, col : col + s],
                                    start=False,
                                    stop=(hc == 1 and m % 2 == 1),
                                )
                    # flush fpre matmuls deferred from the previous block /
                    # level: by now their gate chains have drained, and the
                    # matmuls above kept the PE stream dense in the meantime.
                    # At block 0 of a multi-block level, hold back the LAST
                    # TWO pending thunks: they are the child's final fpre
                    # blocks, whose h was emitted only in the level-start
                    # tails flush, and their consumer is this level's last
                    # pre_f -- deferring them to block 1 avoids stalling the
                    # PE on that fresh chain.  (Older backlog entries feed
                    # pre_f(b0) and must go out now.)
                    if b == 0 and S > NB:
                        while len(pending) > 2:
                            pending.pop(0)()
                    else:
                        for th in pending:
                            th()
                        pending.clear()
                    if not leaf:
                        pre_f = wk.tile([128, 4, NB], bf16, tag="pre_f", name="pre_f")
                        fx_ap = pt[3][:, :, :s]
                        for side in range(2):
                            nc.vector.tensor_add(
                                pre_f[:, 2 * side : 2 * side + 2, :s],
                                f_child[:, :, 2 * col + side : 2 * (col + s) : 2],
                                fx_ap,
                            )
                    # ---- gates (front half) ----
                    # bufs=3: the tail two blocks later still reads the o gate
                    sig_io = wk.tile(
                        [128, 4, NB], f32, tag="sig_io", name="sig_io", bufs=3
                    )
                    u_t = wk.tile([128, 2, NB], f32, tag="u_t", name="u_t")
                    nc.scalar.activation(sig_io[:, 0:2, :s], pt[0][:, :, :s], SIG)
                    nc.scalar.activation(sig_io[:, 2:4, :s], pt[1][:, :, :s], SIG)
                    nc.scalar.activation(u_t[:, :, :s], pt[2][:, :, :s], TANH)
                    # sig_f issued BEFORE the deferred tails so the fc chain
                    # starts as soon as pre_f lands, not after the old tanh
                    sig_f = None
                    if not leaf:
                        sig_f = wk.tile(
                            [128, 4, NB], f32, tag="sig_f", name="sig_f", bufs=1
                        )
                        nc.scalar.activation(sig_f[:, :, :s], pre_f[:, :, :s], SIG)
                    # chain tails deferred TWO blocks run here: their inputs
                    # are long ready, so they never stall the Scalar/DVE
                    # queues between this block's front half and the next's
                    while len(tails) > 1:
                        tails.pop(0)()
                    cs = c_dest[:, :, col : col + s]
                    nc.vector.tensor_mul(cs, sig_io[:, 0:2, :s], u_t[:, :, :s])
                    if not leaf:
                        fc = wk.tile([128, 2, NB], f32, tag="fc", name="fc")
                        nc.vector.tensor_mul(
                            fc[:, :, :s],
                            sig_f[:, 0:2, :s],
                            c_child[:, :, 2 * col : 2 * (col + s) : 2],
                        )
                        nc.vector.tensor_add(cs, cs, fc[:, :, :s])
                        fc2 = wk.tile([128, 2, NB], f32, tag="fc", name="fc2")
                        nc.vector.tensor_mul(
                            fc2[:, :, :s],
                            sig_f[:, 2:4, :s],
                            c_child[:, :, 2 * col + 1 : 2 * (col + s) : 2],
                        )
                        nc.vector.tensor_add(cs, cs, fc2[:, :, :s])

                    # ---- chain tail (tanh, h, hs, fpre): deferred by one
                    # block so consecutive gate chains pipeline instead of
                    # serializing through the in-order Scalar queue ----
                    def tail_thunk(col=col, s=s, cs=cs, sig_io=sig_io, leaf=leaf):
                        tc_t = wk.tile(
                            [128, 2, NB], f32, tag="tc_t", name="tc_t", bufs=1
                        )
                        nc.scalar.activation(tc_t[:, :, :s], cs, TANH)
                        nc.vector.tensor_mul(
                            h_dest[:, :, col : col + s],
                            sig_io[:, 2:4, :s],
                            tc_t[:, :, :s],
                        )
                        # parent's child-h sums for every completed block pair
                        if hs_sink is not None:
                            n_ready = (col + s) // (2 * sp)
                            for j in range(hs_done[0], n_ready):
                                nc.vector.tensor_add(
                                    hs_sink[:, :, j * sp : (j + 1) * sp],
                                    h_dest[:, :, 2 * j * sp : 2 * (j + 1) * sp : 2],
                                    h_dest[
                                        :, :, 2 * j * sp + 1 : 2 * (j + 1) * sp : 2
                                    ],
                                )
                            hs_done[0] = n_ready
                        # fpre for this block: deferred until the parent
                        # needs it
                        if fpre_out is not None:

                            def fpre_thunk():
                                psf = psp.tile([128, 2, NB], f32, tag="ps", name="psf")
                                for m in range(2):
                                    for hc in range(2):
                                        nc.tensor.matmul(
                                            psf[:, m, :s],
                                            wf_sb[hc][:, m * 128 : (m + 1) * 128],
                                            h_dest[:, hc, col : col + s],
                                            start=(m == 0 and hc == 0),
                                            stop=(m == 1 and hc == 1),
                                        )
                                # gpsimd cannot read PSUM: leaf thunks drain
                                # on the DVE, the rest on ScalarE
                                if leaf:
                                    nc.vector.tensor_copy(
                                        fpre_out[:, :, col : col + s], psf[:, :, :s]
                                    )
                                else:
                                    nc.scalar.copy(
                                        fpre_out[:, :, col : col + s], psf[:, :, :s]
                                    )

                            pending.append(fpre_thunk)

                    tails.append(tail_thunk)

                # level outputs: emitted after the last block's tail (which is
                # still in `tails`), so enqueue as a tail thunk of their own.
                # gpsimd is a pure DMA-issue queue, so the level-granular wait
                # here never delays compute ops.
                def out_thunk():
                    for ch in range(2):
                        nc.gpsimd.dma_start(
                            out[ch, :, out_col0 : out_col0 + S], c_dest[:, ch, :S]
                        )
                        nc.gpsimd.dma_start(
                            outh[ch, :, out_col0 : out_col0 + S], h_dest[:, ch, :S]
                        )

                tails.append(out_thunk)
                return pending, tails

            def do_small_level(
                S,
                x_col0,
                out_col0,
                h_child,  # bf16 AP [128, 2, 2S]
                c_child,  # f32 AP [128, 2, 2S]
                h_dest,  # bf16 AP [128, 2, >=S]
                c_dest,  # f32 AP [128, 2, >=S]
                root_sink=None,
            ):
                # Single-block level (S <= 128).  The x-side preactivations
                # accumulate directly in PSUM (emitted early, no input deps,
                # so the PE does them during the previous level's gate chain);
                # the h-side matmuls land on top with start=False and the
                # activations then read PSUM directly -- no DVE pre-adds, no
                # hs sum, no fpre round trip.  Layout: ps_io bank=m%2 offset
                # (m//2)*s -> i at [:, :, 0:s], o at [:, :, s:2s], u at 2s:3s;
                # ps_f bank=f-chunk, offset side*s.
                s = S
                xo = x_col0 - XSB_COL0
                # child-h sum first: halves the iou h-side matmul count
                hs_s = wk.tile([128, 2, 128], bf16, tag="hs_s", name="hs_s")
                nc.vector.tensor_add(
                    hs_s[:, :, :s],
                    h_child[:, :, 0 : 2 * s : 2],
                    h_child[:, :, 1 : 2 * s : 2],
                )
                ps_io = psp.tile([128, 2, NB], f32, tag="ps", name="ps_io")
                ps_f = psp.tile([128, 2, NB], f32, tag="ps", name="ps_f")
                # each ps tile is ONE 2KB bank: exactly one start (the very
                # first matmul into the tile) and one stop (the very last)
                # across both halves and all chunk regions
                for b in range(2):
                    for mi, m in enumerate((b, b + 2, b + 4)):
                        ap = ps_io[:, b, mi * s : (mi + 1) * s]
                        msl = slice(m * 128, (m + 1) * 128)
                        for ki in range(3):
                            nc.tensor.matmul(
                                ap,
                                wx_sb[ki][:, msl],
                                xs_small[ki][:, xo : xo + s],
                                start=(b == 0 and mi == 0 and ki == 0),
                                stop=False,
                            )
                # f gates per child, interleaved: fx from the duplicated-x
                # copy, one pass over 2s columns per chunk
                for m in range(2):
                    msl = slice((6 + m) * 128, (7 + m) * 128)
                    ap = ps_f[:, m, : 2 * s]
                    for ki in range(3):
                        nc.tensor.matmul(
                            ap,
                            wx_sb[ki][:, msl],
                            xs_dup[ki][:, 2 * xo : 2 * (xo + s)],
                            start=(m == 0 and ki == 0),
                            stop=False,
                        )
                # h-side iou on the pre-summed children
                for b in range(2):
                    for mi, m in enumerate((b, b + 2, b + 4)):
                        ap = ps_io[:, b, mi * s : (mi + 1) * s]
                        msl = slice(m * 128, (m + 1) * 128)
                        for hc in range(2):
                            nc.tensor.matmul(
                                ap,
                                wh_sb[hc][:, msl],
                                hs_s[:, hc, :s],
                                start=False,
                                stop=(b == 1 and mi == 2 and hc == 1),
                            )
                # h-side f per child (contiguous interleaved children)
                for m in range(2):
                    ap = ps_f[:, m, : 2 * s]
                    for hc in range(2):
                        nc.tensor.matmul(
                            ap,
                            wf_sb[hc][:, m * 128 : (m + 1) * 128],
                            h_child[:, hc, 0 : 2 * s],
                            start=False,
                            stop=(m == 1 and hc == 1),
                        )
                # ---- gates (activations read PSUM directly) ----
                sig_io = wk.tile([128, 2, 256], f32, tag="sio_s", name="sio_s")
                u_t = wk.tile([128, 2, 128], f32, tag="u_s", name="u_s")
                sig_f = wk.tile([128, 2, 256], f32, tag="sf_s", name="sf_s")
                # sig_f first: the interleaved fc multiply runs on the DVE in
                # parallel with the remaining iou activations
                nc.scalar.activation(sig_f[:, :, : 2 * s], ps_f[:, :, : 2 * s], SIG)
                nc.scalar.activation(sig_io[:, :, : 2 * s], ps_io[:, :, : 2 * s], SIG)
                nc.scalar.activation(u_t[:, :, :s], ps_io[:, :, 2 * s : 3 * s], TANH)
                cs = c_dest[:, :, 0:s]
                # fc for both children in one interleaved multiply, then two
                # strided adds fold them into cs
                fc = wk.tile([128, 2, 128], f32, tag="fc_s", name="fc_s")
                nc.vector.tensor_mul(
                    fc[:, :, : 2 * s], sig_f[:, :, : 2 * s], c_child[:, :, : 2 * s]
                )
                nc.vector.tensor_mul(cs, sig_io[:, :, 0:s], u_t[:, :, :s])
                nc.vector.tensor_add(cs, cs, fc[:, :, 0 : 2 * s : 2])
                nc.vector.tensor_add(cs, cs, fc[:, :, 1 : 2 * s : 2])
                tc_t = wk.tile([128, 2, 128], f32, tag="tc_s", name="tc_s", bufs=1)
                nc.scalar.activation(tc_t[:, :, :s], cs, TANH)
                nc.vector.tensor_mul(
                    h_dest[:, :, 0:s], sig_io[:, :, s : 2 * s], tc_t[:, :, :s]
                )
                if root_sink is not None and S == 1:
                    # stage (c, h) as one f32 tile (DVE casts h) and ship it
                    # with ONE sync DMA so the collective trigger fires after
                    # a single completion
                    gin = root_sink
                    stage = wk.tile([128, 2, 2], f32, tag="gstage", bufs=1)
                    nc.vector.tensor_copy(stage[:, :, 0:1], cs[:, :, 0:1])
                    nc.vector.tensor_copy(stage[:, :, 1:2], h_dest[:, :, 0:1])
                    nc.sync.dma_start(
                        gin[:, :].rearrange("(c p) t -> p c t", p=128),
                        stage[:, :, :],
                    )
                for ch in range(2):
                    nc.sync.dma_start(
                        out[ch, :, out_col0 : out_col0 + S], c_dest[:, ch, :S]
                    )
                    nc.gpsimd.dma_start(
                        outh[ch, :, out_col0 : out_col0 + S], h_dest[:, ch, :S]
                    )

            # ---- gather bounce buffers ----
            gin = dram.tile([256, 2], f32)
            # anti-DCE sink for the warm-up (overwritten by the root DMAs)
            nc.sync.dma_start(gin[0:128, :], jo[:])
            # NOTE: addr_space="Shared" output was tried here and reverted:
            # the shared-output AllGather path measured 102us vs 21us Local.
            gout = dram.tile([256 * NCORES, 2], f32)

            # ---- child-h sum ping-pong buffers (hs for level d lives in
            # hsbuf[d % 2]; produced during level d+1, consumed by level d's
            # h-side matmuls) ----
            hsbuf = {
                0: ppool.tile([128, 2, 2048], bf16, tag="hsA", name="hsA"),
                1: ppool.tile([128, 2, 1024], bf16, tag="hsB", name="hsB"),
            }

            # ---- sharded levels 15..3 ----
            pending = []
            tails = []
            for d in LEVELS:
                S = S_OF[d]
                par = d & 1
                h_child = c_child = f_child = None
                if d < DEPTH:
                    h_child = hbuf[1 - par][:, :, : 2 * S]
                    c_child = cbuf[1 - par][:, :, : 2 * S]
                    f_child = fbuf[1 - par][:, :, : 2 * S]
                if S > SMALL_S:
                    parent_big = S_OF.get(d - 1, 0) > SMALL_S
                    pending, tails = do_level(
                        S,
                        OFF_OF[d],
                        OFF_OF[d],
                        hsbuf[d % 2][:, :, :S] if d < DEPTH else None,
                        c_child,
                        f_child,
                        hbuf[par],
                        cbuf[par],
                        # parent uses the fpre path only while it is itself
                        # a big level
                        fbuf[par][:, :, :S] if parent_big else None,
                        hs_sink=(
                            hsbuf[(d - 1) % 2][:, :, : S // 2] if parent_big else None
                        ),
                        pending=pending,
                        tails=tails,
                    )
                    if d == 13:
                        load_xs_small()
                    if d == 11:
                        # second warm-up collective shortly before the real
                        # gather: keeps ncfw hot and realigns the cores (a
                        # barrier), cutting the real gather's arrival skew.
                        # Its input reads this level's just-computed c so it
                        # cannot fire earlier than ~15us before the root.
                        gin_w2 = dram.tile([32, 1], f32)
                        # col 128: live L11 data, but past the columns the
                        # later small levels overwrite (no WAR stall)
                        nc.sync.dma_start(gin_w2[:, :], cbuf[par][0:32, 0, 128:129])
                        gout_w2 = dram.tile([32 * NCORES, 1], f32)
                        nc.gpsimd.collective_compute(
                            "AllGather",
                            mybir.AluOpType.bypass,
                            replica_groups=[list(range(NCORES))],
                            ins=[gin_w2.opt()],
                            outs=[gout_w2.opt()],
                        )
                else:
                    for th in tails:
                        th()
                    tails = []
                    for th in pending:
                        th()
                    pending = []
                    do_small_level(
                        S,
                        OFF_OF[d],
                        OFF_OF[d],
                        h_child,
                        c_child,
                        hbuf[par],
                        cbuf[par],
                        root_sink=gin if d == SUB_DEPTH else None,
                    )

            # ---- allgather the 8 subtree roots ----
            nc.gpsimd.collective_compute(
                "AllGather",
                mybir.AluOpType.bypass,
                replica_groups=[list(range(NCORES))],
                ins=[gin.opt()],
                outs=[gout.opt()],
            )
            # load gathered roots feature-major: (p, ch, rank); h loads via a
            # casting DMA (gpsimd) straight to bf16 -- no SBUF round trip
            g_ap = gout[:, :].rearrange("(r c p) t -> c p r t", p=128, c=2)
            c_top = ppool.tile([128, 2, 8], f32, tag="ctop")
            h_top = ppool.tile([128, 2, 8], bf16, tag="htop")
            for ch in range(2):
                nc.sync.dma_start(c_top[:, ch, :], g_ap[ch, :, :, 0])
                nc.gpsimd.dma_start(h_top[:, ch, :], g_ap[ch, :, :, 1])

            # ---- top levels 2..0 (computed redundantly on every core) ----
            prev_h, prev_c = h_top, c_top
            for d in (2, 1, 0):
                S = 2**d
                node0 = S - 1
                col0 = TOP_COL0 + node0
                h_d = ppool.tile([128, 2, S], bf16, tag=f"ht{d}", name=f"ht{d}")
                c_d = ppool.tile([128, 2, S], f32, tag=f"ct{d}", name=f"ct{d}")
                do_small_level(
                    S,
                    col0,
                    col0,
                    prev_h[:, :, : 2 * S],
                    prev_c[:, :, : 2 * S],
                    h_d,
                    c_d,
                )
                prev_h, prev_c = h_d, c_d

    nc.compile()
    return nc


def _get_program():
    if "nc" not in _CACHE:
        _CACHE["nc"] = _build_program()
    return _CACHE["nc"]


def _preprocess(inputs, W_ioux, b_ioux, W_iouh, b_iouh, W_fx, b_fx, W_fh, b_fh):
    """Build per-core input maps (numpy only)."""
    bf = ml_dtypes.bfloat16
    wx_cat = np.concatenate([W_ioux, W_fx], axis=0)  # [1024, 300]
    b_cat = np.concatenate([b_ioux + b_iouh, b_fx + b_fh], axis=0)  # [1024]
    # K padded to 384 with zero weight rows (keeps FWL on for every matmul)
    wx_full = np.zeros((K_PAD, 4 * MEM), np.float32)
    wx_full[:IN_DIM] = wx_cat.T
    wx_full[IN_DIM] = b_cat
    wx_np = np.ascontiguousarray(wx_full).astype(bf)
    wh_np = np.ascontiguousarray(W_iouh.T).astype(bf)  # [256, 768]
    wf_np = np.ascontiguousarray(W_fh.T).astype(bf)  # [256, 256]

    xT = np.zeros((K_PAD, N_NODES), np.float32)
    xT[:IN_DIM] = inputs.T
    xT[IN_DIM] = 1.0
    xT = xT.astype(bf)  # [384, 65535]

    in_maps = []
    for j in range(NCORES):
        segs = []
        for d in LEVELS:
            S = S_OF[d]
            g0 = (2**d - 1) + j * S
            segs.append(np.arange(g0, g0 + S))
        segs.append(np.arange(0, 7))
        cols = np.concatenate(segs)
        xcore = np.ascontiguousarray(xT[:, cols])
        # duplicated columns of the small-level x range (per-child f gates)
        xdup = np.ascontiguousarray(np.repeat(xcore[:, XSB_COL0:], 2, axis=1))
        in_maps.append(
            {"xt": xcore, "xdup": xdup, "wx": wx_np, "wh": wh_np, "wf": wf_np}
        )
    return in_maps


def _postprocess(results):
    """Assemble [2, N, 256] from per-core outputs (c from `out` row 0, h from
    the bf16 `outh`, upcast on host)."""
    full = np.empty((2, N_NODES, MEM), np.float32)
    for j in range(NCORES):
        rc = results[j]["out"]  # [2(ch), 128, N_COLS] f32
        rh = results[j]["outh"].astype(np.float32)  # [2(ch), 128, N_COLS]
        r = np.stack([rc, rh])  # [2(c/h), 2, 128, N_COLS]
        for d in LEVELS:
            S = S_OF[d]
            g0 = (2**d - 1) + j * S
            off = OFF_OF[d]
            blk = r[:, :, :, off : off + S]  # [2,2,128,S]
            full[:, g0 : g0 + S, :] = blk.transpose(0, 3, 1, 2).reshape(2, S, MEM)
        if j == 0:
            r0 = r[:, :, :, TOP_COL0 : TOP_COL0 + 7]
            full[:, 0:7, :] = r0.transpose(0, 3, 1, 2).reshape(2, 7, MEM)
    return full


def kernel(**inputs):
    from concourse.bass_utils import run_bass_kernel_spmd

    nc = _get_program()
    inputs = {k: np.asarray(v) for k, v in inputs.items()}
    in_maps = _preprocess(**inputs)
    res = run_bass_kernel_spmd(nc, in_maps, core_ids=list(range(NCORES)))
    _CACHE["last_result"] = res
    return _postprocess(res.results)



# revision 73
# speedup vs baseline: 1.2500x; 1.2500x over previous
"""Child-Sum TreeLSTM over a perfect binary tree (N=65535, depth 15) on 8 trn2 cores.

Sharding: each core owns one depth-3 subtree (levels 15..3 are fully local:
children of node range [a,b) are [2a+1,2b+1), so per-core level slices are
contiguous and child gathers are stride-2 local reads).  The 8 subtree roots
are AllGathered (16 KB) and the top 7 nodes are computed redundantly on every
core; the host takes them from core 0.

On-chip layout is feature-major ([feature-chunk=128 partitions, nodes free]);
the host pre-transposes the inputs so the device never transposes anything.
Biases are folded into the x-side matmul via an appended ones-row; K is
padded 301 -> 384 because matmuls with <128-partition weights run at half
the sustained PE rate (no FWL: hw-measured 427 vs 216 ns per 512-col MM).
Matmuls run in bf16 (fp32 PSUM accumulation); cell state is fp32.

Big levels run in 256-column blocks; every psum tile is exactly ONE 2KB bank
([2 banks-halves, 256] f32 holding an m-chunk pair), so a block holds 4-5 of
the 8 banks and ~2 blocks pipeline through PSUM.  The gate-chain tail (tanh,
h-mul, child-h sums, fpre enqueue) of each block is deferred by TWO blocks so
consecutive chains pipeline instead of serializing through the in-order
Scalar queue.  Child-h sums (hs) for the next level are produced on the DVE
as soon as each source block pair's h lands, into ping-pong hsbuf buffers,
so the parent's h-side matmuls never wait.  The f-gate h-side matmuls (fpre)
are deferred thunks flushed after the NEXT block's dense matmuls.  x arrives
via [128, 1024] supertiles prefetched 2 ahead on the sync queue; outputs
leave per level on the gpsimd queue (pure DMA-issue, so its waits block no
compute).  Two junk warm-up AllGathers run before the real root gather: the
first absorbs ncfw cold start, the second (input-gated on level-11 data)
fires ~15us before the root and realigns the cores, cutting arrival skew.
The serial small-level tail (S<=64) batches its x-side into persistent tiles
(plus a column-duplicated copy so per-child f-gates are single interleaved
passes) and reads PSUM directly from the activations.
"""

import sys

sys.path.insert(0, "/opt/trn_rl_repo")

import numpy as np
import ml_dtypes

IN_DIM = 300
MEM = 256
DEPTH = 15
N_NODES = 2 ** (DEPTH + 1) - 1  # 65535
NCORES = 8
SUB_DEPTH = 3  # shard at depth 3 -> 8 subtrees
NB = 256  # node block size: [2, 256] f32 = one 2KB psum bank per m-pair tile,
# so a block holds 4-5 of 8 banks and ~2 blocks pipeline through PSUM
SMALL_S = 64  # levels with S <= this use the batched x pass (3*S <= NB)
XSUP = 1024  # x prefetch supertile width (columns)
# K padded 301 -> 384: matmuls with <128-partition weights run at half the
# sustained PE rate (no FWL; hw-measured 427 vs 216 ns per 512-col MM), so
# the partial 45-row chunk is padded with zero weight rows to keep the whole
# stream warm.
K_PAD = 384

LEVELS = list(range(DEPTH, SUB_DEPTH - 1, -1))  # 15..3
S_OF = {d: 2 ** (d - SUB_DEPTH) for d in LEVELS}  # 4096..1
OFF_OF = {}
_off = 0
for _d in LEVELS:
    OFF_OF[_d] = _off
    _off += S_OF[_d]
N_LOCAL = _off  # 8191
TOP_COL0 = N_LOCAL  # columns 8191..8197 hold x of global nodes 0..6
N_COLS = N_LOCAL + 7  # 8198
_batch_levels = [d for d in LEVELS if S_OF[d] <= SMALL_S]
XSB_COL0 = OFF_OF[_batch_levels[0]]  # first column served by the batched x pass
XSB_N = N_COLS - XSB_COL0  # 134

_CACHE = {}


def _ceil_div(a, b):
    return -(-a // b)


def _build_program():
    import concourse.mybir as mybir
    import concourse.bacc as bacc
    from concourse import tile

    f32 = mybir.dt.float32
    bf16 = mybir.dt.bfloat16
    f8 = mybir.dt.float8e4
    DR = mybir.MatmulPerfMode.DoubleRow
    SIG = mybir.ActivationFunctionType.Sigmoid
    TANH = mybir.ActivationFunctionType.Tanh

    nc = bacc.Bacc("TRN2", target_bir_lowering=False, debug=False, num_devices=NCORES)

    # NOTE: fp8 DoubleRow for the x-side was tried and reverted: the HAM
    # power-throttles the PE to 4/8 clock while fp8 runs (min matmul dur
    # 426ns = 2x the bf16 floor), erasing the throughput gain for the whole
    # stream.
    xt = nc.dram_tensor("xt", [K_PAD, N_COLS], bf16, kind="ExternalInput")
    xdup = nc.dram_tensor("xdup", [K_PAD, 2 * XSB_N], bf16, kind="ExternalInput")
    wx = nc.dram_tensor("wx", [K_PAD, 4 * MEM], bf16, kind="ExternalInput")
    wh = nc.dram_tensor("wh", [MEM, 3 * MEM], bf16, kind="ExternalInput")
    wf = nc.dram_tensor("wf", [MEM, MEM], bf16, kind="ExternalInput")
    out = nc.dram_tensor("out", [2, 128, N_COLS], f32, kind="ExternalOutput")
    # h written in its native bf16 (the host upcasts): 25% less output DMA
    outh = nc.dram_tensor("outh", [2, 128, N_COLS], bf16, kind="ExternalOutput")

    KCH = [(0, 128), (128, 128), (256, 128)]  # k chunks of K_PAD=384

    with tile.TileContext(nc) as tc:
        with (
            tc.tile_pool(name="const", bufs=1) as cpool,
            tc.tile_pool(name="perst", bufs=1) as ppool,
            tc.tile_pool(name="xp", bufs=4) as xpool,
            tc.tile_pool(name="wk", bufs=2) as wk,
            tc.tile_pool(name="ps", bufs=8, space="PSUM") as psp,
            tc.tile_pool(name="dram", bufs=1, space="DRAM") as dram,
        ):
            # ---- PE warm-up: dense junk matmuls with no input deps run
            # during the initial DMA window so the HAM un-throttles the PE
            # clock (4/8 -> 8/8) before the real matmul stream begins
            jw = wk.tile([128, 128], bf16, tag="jw", name="jw", bufs=1)
            jx = wk.tile([128, NB], bf16, tag="jx", name="jx", bufs=1)
            nc.vector.memset(jw[:], 0.0)
            nc.vector.memset(jx[:], 0.0)
            pw = [
                psp.tile([128, 2, NB], f32, tag="ps", name=f"pw{j}") for j in range(2)
            ]
            # enough reps to bridge the whole startup DMA window (~13us) so
            # the PE stays warm until the first supertile lands
            for i in range(96):
                nc.tensor.matmul(
                    pw[(i // 2) % 2][:, i % 2, :],
                    jw[:],
                    jx[:],
                    start=True,
                    stop=True,
                )
            # consume the warm-up results so they are not dead-code-eliminated
            # (gin is fully overwritten by the real root DMAs later)
            jo = wk.tile([128, 2], f32, tag="jo", name="jo", bufs=1)
            nc.vector.tensor_copy(jo[:], pw[0][:, 0, 0:2])
            nc.vector.tensor_copy(jo[:], pw[1][:, 1, 0:2])

            # ---- load weights ----
            wx_sb = []
            for i, (k0, kn) in enumerate(KCH):
                t = cpool.tile([kn, 4 * MEM], bf16, tag=f"wx{i}", name=f"wx{i}")
                # quarter-column pieces: short per-engine chains at startup
                for q in range(4):
                    c0, c1 = q * MEM, (q + 1) * MEM
                    nc.sync.dma_start(t[:, c0:c1], wx[k0 : k0 + kn, c0:c1])
                wx_sb.append(t)

            # ---- x supertile prefetcher: [128, 1024]-col loads keep the
            # sync-queue issue count low and prefetch 2 supertiles ahead.
            # The first supertile is issued right after the wx loads so the
            # first leaf matmuls aren't starved behind wh/wf/xs loads. ----
            BIG_COLS = OFF_OF[_batch_levels[0]]  # big-level x range [0, BIG_COLS)
            N_SUP = _ceil_div(BIG_COLS, XSUP)
            sup_tiles = [None] * N_SUP
            sup_state = {"next": 0}

            def _load_sup(k):
                c0 = k * XSUP
                cn = min(XSUP, BIG_COLS - c0)
                ts_ = []
                for i, (k0, kn) in enumerate(KCH):
                    t = xpool.tile([kn, XSUP], bf16, tag=f"xk{i}", name=f"xs{k}_{i}")
                    # two half-column DMAs: twice the engines per supertile
                    h = cn // 2
                    nc.sync.dma_start(t[:, :h], xt[k0 : k0 + kn, c0 : c0 + h])
                    nc.sync.dma_start(
                        t[:, h:cn], xt[k0 : k0 + kn, c0 + h : c0 + cn]
                    )
                    ts_.append(t)
                sup_tiles[k] = ts_

            def get_x(col0):
                k = col0 // XSUP
                want = min(k + 3, N_SUP - 1)
                while sup_state["next"] <= want:
                    _load_sup(sup_state["next"])
                    sup_state["next"] += 1
                return sup_tiles[k], col0 - k * XSUP

            # first block's 256 columns load in small dedicated pieces ahead
            # of everything else so the first real matmuls start early
            sup0 = []
            for i, (k0, kn) in enumerate(KCH):
                t = xpool.tile([kn, XSUP], bf16, tag=f"xk{i}", name=f"xs0_{i}")
                nc.sync.dma_start(t[:, :NB], xt[k0 : k0 + kn, 0:NB])
                sup0.append(t)
            for i, (k0, kn) in enumerate(KCH):
                t = sup0[i]
                nc.sync.dma_start(t[:, NB:640], xt[k0 : k0 + kn, NB:640])
                nc.sync.dma_start(t[:, 640:XSUP], xt[k0 : k0 + kn, 640:XSUP])
            sup_tiles[0] = sup0
            sup_state["next"] = 1

            wh_sb = []
            for i in range(2):
                t = cpool.tile([128, 3 * MEM], bf16, tag=f"wh{i}", name=f"wh{i}")
                nc.sync.dma_start(t[:], wh[i * 128 : (i + 1) * 128, :])
                wh_sb.append(t)
            wf_sb = []
            for i in range(2):
                t = cpool.tile([128, MEM], bf16, tag=f"wf{i}", name=f"wf{i}")
                nc.sync.dma_start(t[:], wf[i * 128 : (i + 1) * 128, :])
                wf_sb.append(t)

            # ---- warm-up collective: a junk 1KB AllGather issued up front
            # keeps ncfw/the CC stream warm so the real root gather at the
            # end does not pay the cold-start trigger latency.
            gin_w = dram.tile([32, 1], f32)
            nc.sync.dma_start(gin_w[:, :], jo[0:32, 0:1])
            gout_w = dram.tile([32 * NCORES, 1], f32)
            nc.gpsimd.collective_compute(
                "AllGather",
                mybir.AluOpType.bypass,
                replica_groups=[list(range(NCORES))],
                ins=[gin_w.opt()],
                outs=[gout_w.opt()],
            )

            # ---- persistent level buffers (A = odd levels, B = even) ----
            hbuf = {
                1: ppool.tile([128, 2, 4096], bf16, tag="hA", name="hA"),
                0: ppool.tile([128, 2, 2048], bf16, tag="hB", name="hB"),
            }
            cbuf = {
                1: ppool.tile([128, 2, 4096], f32, tag="cA", name="cA"),
                0: ppool.tile([128, 2, 2048], f32, tag="cB", name="cB"),
            }
            fbuf = {
                1: ppool.tile([128, 2, 4096], bf16, tag="fA", name="fA"),
                0: ppool.tile([128, 2, 2048], bf16, tag="fB", name="fB"),
            }

            # ---- persistent x tiles for the tiny levels + top (134 cols),
            # plus a column-duplicated copy for the per-child f-gate pass.
            # Loads are EMITTED mid-kernel (at level 13) so the startup DMA
            # burst doesn't delay the leaf-phase supertiles. ----
            xs_small = []
            xs_dup = []
            for i, (k0, kn) in enumerate(KCH):
                xs_small.append(
                    cpool.tile([kn, XSB_N], bf16, tag=f"xs{i}", name=f"xs{i}")
                )
                xs_dup.append(
                    cpool.tile([kn, 2 * XSB_N], bf16, tag=f"xd{i}", name=f"xd{i}")
                )

            def load_xs_small():
                for i, (k0, kn) in enumerate(KCH):
                    nc.sync.dma_start(
                        xs_small[i][:], xt[k0 : k0 + kn, XSB_COL0:N_COLS]
                    )
                    nc.sync.dma_start(xs_dup[i][:], xdup[k0 : k0 + kn, :])

            def do_level(
                S,
                x_col0,
                out_col0,
                hs_src,  # bf16 AP [128, 2, S]: precomputed child-h sums, or None (leaf)
                c_child,  # f32 AP [128, 2, 2S] or None
                f_child,  # bf16 AP [128, 2, 2S] (fpre of children) or None
                h_dest,  # bf16 AP [128, 2, >=S]
                c_dest,  # f32 AP [128, 2, >=S]
                fpre_out,  # bf16 AP [128, 2, >=S] or None
                hs_sink=None,  # bf16 AP [128, 2, S//2]: this level's child-h
                # sums for the parent, produced as soon as each source block
                # pair's h lands (keeps the parent's h-side matmuls unblocked)
                pending=None,  # deferred fpre-matmul thunks (cross-level)
                tails=None,  # deferred gate-chain tails (cross-level)
            ):
                leaf = c_child is None
                sp = min(NB, S // 2) if hs_sink is not None else 0
                hs_done = [0]
                # the child level's last tails still sit in `tails`; they
                # produce the final hs chunk and fpre blocks this level's
                # matmuls read, so emit them before the block loop
                for th in tails:
                    th()
                tails.clear()
                for b in range(_ceil_div(S, NB)):
                    col = b * NB
                    s = min(NB, S - col)
                    n_m = 6 if leaf else 8
                    xts, xo = get_x(x_col0 + col)
                    pt = [
                        psp.tile([128, 2, NB], f32, tag="ps", name=f"pt{j}")
                        for j in range(n_m // 2)
                    ]
                    # chunk-major m-pairs: each pt[j] is ONE psum bank holding
                    # m=2j and m=2j+1; exactly one start (first matmul into
                    # the bank) and one stop (last matmul into the bank)
                    for m in range(n_m):
                        msl = slice(m * 128, (m + 1) * 128)
                        lastx = m % 2 == 1 and (leaf or m == 7)
                        for ki in range(3):
                            nc.tensor.matmul(
                                pt[m // 2][:, m % 2, :s],
                                wx_sb[ki][:, msl],
                                xts[ki][:, xo : xo + s],
                                start=(ki == 0 and m % 2 == 0),
                                stop=(ki == 2 and lastx),
                            )
                        if not leaf and m < 6:
                            for hc in range(2):
                                nc.tensor.matmul(
                                    pt[m // 2][:, m % 2, :s],
                                    wh_sb[hc][:, msl],
                                    hs_src[:, hc, col : col + s],
                                    start=False,
                                    stop=(hc == 1 and m % 2 == 1),
                                )
                    # flush fpre matmuls deferred from the previous block /
                    # level: by now their gate chains have drained, and the
                    # matmuls above kept the PE stream dense in the meantime.
                    # At block 0 of a multi-block level, hold back the LAST
                    # TWO pending thunks: they are the child's final fpre
                    # blocks, whose h was emitted only in the level-start
                    # tails flush, and their consumer is this level's last
                    # pre_f -- deferring them to block 1 avoids stalling the
                    # PE on that fresh chain.  (Older backlog entries feed
                    # pre_f(b0) and must go out now.)
                    if b == 0 and S > NB:
                        while len(pending) > 2:
                            pending.pop(0)()
                    else:
                        for th in pending:
                            th()
                        pending.clear()
                    if not leaf:
                        pre_f = wk.tile([128, 4, NB], bf16, tag="pre_f", name="pre_f")
                        fx_ap = pt[3][:, :, :s]
                        for side in range(2):
                            nc.vector.tensor_add(
                                pre_f[:, 2 * side : 2 * side + 2, :s],
                                f_child[:, :, 2 * col + side : 2 * (col + s) : 2],
                                fx_ap,
                            )
                    # ---- gates (front half) ----
                    # bufs=3: the tail two blocks later still reads the o gate
                    sig_io = wk.tile(
                        [128, 4, NB], f32, tag="sig_io", name="sig_io", bufs=3
                    )
                    u_t = wk.tile([128, 2, NB], f32, tag="u_t", name="u_t")
                    nc.scalar.activation(sig_io[:, 0:2, :s], pt[0][:, :, :s], SIG)
                    nc.scalar.activation(sig_io[:, 2:4, :s], pt[1][:, :, :s], SIG)
                    nc.scalar.activation(u_t[:, :, :s], pt[2][:, :, :s], TANH)
                    # sig_f issued BEFORE the deferred tails so the fc chain
                    # starts as soon as pre_f lands, not after the old tanh
                    sig_f = None
                    if not leaf:
                        sig_f = wk.tile(
                            [128, 4, NB], f32, tag="sig_f", name="sig_f", bufs=1
                        )
                        nc.scalar.activation(sig_f[:, :, :s], pre_f[:, :, :s], SIG)
                    # chain tails deferred TWO blocks run here: their inputs
                    # are long ready, so they never stall the Scalar/DVE
                    # queues between this block's front half and the next's
                    while len(tails) > 1:
                        tails.pop(0)()
                    cs = c_dest[:, :, col : col + s]
                    nc.vector.tensor_mul(cs, sig_io[:, 0:2, :s], u_t[:, :, :s])
                    if not leaf:
                        fc = wk.tile([128, 2, NB], f32, tag="fc", name="fc")
                        nc.vector.tensor_mul(
                            fc[:, :, :s],
                            sig_f[:, 0:2, :s],
                            c_child[:, :, 2 * col : 2 * (col + s) : 2],
                        )
                        nc.vector.tensor_add(cs, cs, fc[:, :, :s])
                        fc2 = wk.tile([128, 2, NB], f32, tag="fc", name="fc2")
                        nc.vector.tensor_mul(
                            fc2[:, :, :s],
                            sig_f[:, 2:4, :s],
                            c_child[:, :, 2 * col + 1 : 2 * (col + s) : 2],
                        )
                        nc.vector.tensor_add(cs, cs, fc2[:, :, :s])

                    # ---- chain tail (tanh, h, hs, fpre): deferred by one
                    # block so consecutive gate chains pipeline instead of
                    # serializing through the in-order Scalar queue ----
                    def tail_thunk(col=col, s=s, cs=cs, sig_io=sig_io, leaf=leaf):
                        tc_t = wk.tile(
                            [128, 2, NB], f32, tag="tc_t", name="tc_t", bufs=1
                        )
                        nc.scalar.activation(tc_t[:, :, :s], cs, TANH)
                        nc.vector.tensor_mul(
                            h_dest[:, :, col : col + s],
                            sig_io[:, 2:4, :s],
                            tc_t[:, :, :s],
                        )
                        # parent's child-h sums for every completed block pair
                        if hs_sink is not None:
                            n_ready = (col + s) // (2 * sp)
                            for j in range(hs_done[0], n_ready):
                                nc.vector.tensor_add(
                                    hs_sink[:, :, j * sp : (j + 1) * sp],
                                    h_dest[:, :, 2 * j * sp : 2 * (j + 1) * sp : 2],
                                    h_dest[
                                        :, :, 2 * j * sp + 1 : 2 * (j + 1) * sp : 2
                                    ],
                                )
                            hs_done[0] = n_ready
                        # fpre for this block: deferred until the parent
                        # needs it
                        if fpre_out is not None:

                            def fpre_thunk():
                                psf = psp.tile([128, 2, NB], f32, tag="ps", name="psf")
                                for m in range(2):
                                    for hc in range(2):
                                        nc.tensor.matmul(
                                            psf[:, m, :s],
                                            wf_sb[hc][:, m * 128 : (m + 1) * 128],
                                            h_dest[:, hc, col : col + s],
                                            start=(m == 0 and hc == 0),
                                            stop=(m == 1 and hc == 1),
                                        )
                                # gpsimd cannot read PSUM: leaf thunks drain
                                # on the DVE, the rest on ScalarE
                                if leaf:
                                    nc.vector.tensor_copy(
                                        fpre_out[:, :, col : col + s], psf[:, :, :s]
                                    )
                                else:
                                    nc.scalar.copy(
                                        fpre_out[:, :, col : col + s], psf[:, :, :s]
                                    )

                            pending.append(fpre_thunk)

                    tails.append(tail_thunk)

                # level outputs: emitted after the last block's tail (which is
                # still in `tails`), so enqueue as a tail thunk of their own.
                # gpsimd is a pure DMA-issue queue, so the level-granular wait
                # here never delays compute ops.
                def out_thunk():
                    for ch in range(2):
                        nc.gpsimd.dma_start(
                            out[ch, :, out_col0 : out_col0 + S], c_dest[:, ch, :S]
                        )
                        nc.gpsimd.dma_start(
                            outh[ch, :, out_col0 : out_col0 + S], h_dest[:, ch, :S]
                        )

                tails.append(out_thunk)
                return pending, tails

            def do_small_level(
                S,
                x_col0,
                out_col0,
                h_child,  # bf16 AP [128, 2, 2S]
                c_child,  # f32 AP [128, 2, 2S]
                h_dest,  # bf16 AP [128, 2, >=S]
                c_dest,  # f32 AP [128, 2, >=S]
                root_sink=None,
            ):
                # Single-block level (S <= 128).  The x-side preactivations
                # accumulate directly in PSUM (emitted early, no input deps,
                # so the PE does them during the previous level's gate chain);
                # the h-side matmuls land on top with start=False and the
                # activations then read PSUM directly -- no DVE pre-adds, no
                # hs sum, no fpre round trip.  Layout: ps_io bank=m%2 offset
                # (m//2)*s -> i at [:, :, 0:s], o at [:, :, s:2s], u at 2s:3s;
                # ps_f bank=f-chunk, offset side*s.
                s = S
                xo = x_col0 - XSB_COL0
                # child-h sum first: halves the iou h-side matmul count
                hs_s = wk.tile([128, 2, 128], bf16, tag="hs_s", name="hs_s")
                nc.vector.tensor_add(
                    hs_s[:, :, :s],
                    h_child[:, :, 0 : 2 * s : 2],
                    h_child[:, :, 1 : 2 * s : 2],
                )
                ps_io = psp.tile([128, 2, NB], f32, tag="ps", name="ps_io")
                ps_f = psp.tile([128, 2, NB], f32, tag="ps", name="ps_f")
                # each ps tile is ONE 2KB bank: exactly one start (the very
                # first matmul into the tile) and one stop (the very last)
                # across both halves and all chunk regions
                for b in range(2):
                    for mi, m in enumerate((b, b + 2, b + 4)):
                        ap = ps_io[:, b, mi * s : (mi + 1) * s]
                        msl = slice(m * 128, (m + 1) * 128)
                        for ki in range(3):
                            nc.tensor.matmul(
                                ap,
                                wx_sb[ki][:, msl],
                                xs_small[ki][:, xo : xo + s],
                                start=(b == 0 and mi == 0 and ki == 0),
                                stop=False,
                            )
                # f gates per child, interleaved: fx from the duplicated-x
                # copy, one pass over 2s columns per chunk
                for m in range(2):
                    msl = slice((6 + m) * 128, (7 + m) * 128)
                    ap = ps_f[:, m, : 2 * s]
                    for ki in range(3):
                        nc.tensor.matmul(
                            ap,
                            wx_sb[ki][:, msl],
                            xs_dup[ki][:, 2 * xo : 2 * (xo + s)],
                            start=(m == 0 and ki == 0),
                            stop=False,
                        )
                # h-side iou on the pre-summed children
                for b in range(2):
                    for mi, m in enumerate((b, b + 2, b + 4)):
                        ap = ps_io[:, b, mi * s : (mi + 1) * s]
                        msl = slice(m * 128, (m + 1) * 128)
                        for hc in range(2):
                            nc.tensor.matmul(
                                ap,
                                wh_sb[hc][:, msl],
                                hs_s[:, hc, :s],
                                start=False,
                                stop=(b == 1 and mi == 2 and hc == 1),
                            )
                # h-side f per child (contiguous interleaved children)
                for m in range(2):
                    ap = ps_f[:, m, : 2 * s]
                    for hc in range(2):
                        nc.tensor.matmul(
                            ap,
                            wf_sb[hc][:, m * 128 : (m + 1) * 128],
                            h_child[:, hc, 0 : 2 * s],
                            start=False,
                            stop=(m == 1 and hc == 1),
                        )
                # ---- gates (activations read PSUM directly) ----
                sig_io = wk.tile([128, 2, 256], f32, tag="sio_s", name="sio_s")
                u_t = wk.tile([128, 2, 128], f32, tag="u_s", name="u_s")
                sig_f = wk.tile([128, 2, 256], f32, tag="sf_s", name="sf_s")
                # sig_f first: the interleaved fc multiply runs on the DVE in
                # parallel with the remaining iou activations
                nc.scalar.activation(sig_f[:, :, : 2 * s], ps_f[:, :, : 2 * s], SIG)
                nc.scalar.activation(sig_io[:, :, : 2 * s], ps_io[:, :, : 2 * s], SIG)
                nc.scalar.activation(u_t[:, :, :s], ps_io[:, :, 2 * s : 3 * s], TANH)
                cs = c_dest[:, :, 0:s]
                # fc for both children in one interleaved multiply, then two
                # strided adds fold them into cs
                fc = wk.tile([128, 2, 128], f32, tag="fc_s", name="fc_s")
                nc.vector.tensor_mul(
                    fc[:, :, : 2 * s], sig_f[:, :, : 2 * s], c_child[:, :, : 2 * s]
                )
                nc.vector.tensor_mul(cs, sig_io[:, :, 0:s], u_t[:, :, :s])
                nc.vector.tensor_add(cs, cs, fc[:, :, 0 : 2 * s : 2])
                nc.vector.tensor_add(cs, cs, fc[:, :, 1 : 2 * s : 2])
                tc_t = wk.tile([128, 2, 128], f32, tag="tc_s", name="tc_s", bufs=1)
                nc.scalar.activation(tc_t[:, :, :s], cs, TANH)
                nc.vector.tensor_mul(
                    h_dest[:, :, 0:s], sig_io[:, :, s : 2 * s], tc_t[:, :, :s]
                )
                if root_sink is not None and S == 1:
                    # stage (c, h) as one f32 tile (DVE casts h) and ship it
                    # with ONE sync DMA so the collective trigger fires after
                    # a single completion
                    gin = root_sink
                    stage = wk.tile([128, 2, 2], f32, tag="gstage", bufs=1)
                    nc.vector.tensor_copy(stage[:, :, 0:1], cs[:, :, 0:1])
                    nc.vector.tensor_copy(stage[:, :, 1:2], h_dest[:, :, 0:1])
                    nc.sync.dma_start(
                        gin[:, :].rearrange("(c p) t -> p c t", p=128),
                        stage[:, :, :],
                    )
                for ch in range(2):
                    nc.sync.dma_start(
                        out[ch, :, out_col0 : out_col0 + S], c_dest[:, ch, :S]
                    )
                    nc.gpsimd.dma_start(
                        outh[ch, :, out_col0 : out_col0 + S], h_dest[:, ch, :S]
                    )

            # ---- gather bounce buffers ----
            gin = dram.tile([256, 2], f32)
            # anti-DCE sink for the warm-up (overwritten by the root DMAs)
            nc.sync.dma_start(gin[0:128, :], jo[:])
            # NOTE: addr_space="Shared" output was tried here and reverted:
            # the shared-output AllGather path measured 102us vs 21us Local.
            gout = dram.tile([256 * NCORES, 2], f32)

            # ---- child-h sum ping-pong buffers (hs for level d lives in
            # hsbuf[d % 2]; produced during level d+1, consumed by level d's
            # h-side matmuls) ----
            hsbuf = {
                0: ppool.tile([128, 2, 2048], bf16, tag="hsA", name="hsA"),
                1: ppool.tile([128, 2, 1024], bf16, tag="hsB", name="hsB"),
            }

            # ---- sharded levels 15..3 ----
            pending = []
            tails = []
            for d in LEVELS:
                S = S_OF[d]
                par = d & 1
                h_child = c_child = f_child = None
                if d < DEPTH:
                    h_child = hbuf[1 - par][:, :, : 2 * S]
                    c_child = cbuf[1 - par][:, :, : 2 * S]
                    f_child = fbuf[1 - par][:, :, : 2 * S]
                if S > SMALL_S:
                    parent_big = S_OF.get(d - 1, 0) > SMALL_S
                    pending, tails = do_level(
                        S,
                        OFF_OF[d],
                        OFF_OF[d],
                        hsbuf[d % 2][:, :, :S] if d < DEPTH else None,
                        c_child,
                        f_child,
                        hbuf[par],
                        cbuf[par],
                        # parent uses the fpre path only while it is itself
                        # a big level
                        fbuf[par][:, :, :S] if parent_big else None,
                        hs_sink=(
                            hsbuf[(d - 1) % 2][:, :, : S // 2] if parent_big else None
                        ),
                        pending=pending,
                        tails=tails,
                    )
                    if d == 13:
                        load_xs_small()
                    if d == 11:
                        # second warm-up collective shortly before the real
                        # gather: keeps ncfw hot and realigns the cores (a
                        # barrier), cutting the real gather's arrival skew.
                        # Its input reads this level's just-computed c so it
                        # cannot fire earlier than ~15us before the root.
                        gin_w2 = dram.tile([32, 1], f32)
                        # col 128: live L11 data, but past the columns the
                        # later small levels overwrite (no WAR stall)
                        nc.sync.dma_start(gin_w2[:, :], cbuf[par][0:32, 0, 128:129])
                        gout_w2 = dram.tile([32 * NCORES, 1], f32)
                        nc.gpsimd.collective_compute(
                            "AllGather",
                            mybir.AluOpType.bypass,
                            replica_groups=[list(range(NCORES))],
                            ins=[gin_w2.opt()],
                            outs=[gout_w2.opt()],
                        )
                else:
                    for th in tails:
                        th()
                    tails = []
                    for th in pending:
                        th()
                    pending = []
                    do_small_level(
                        S,
                        OFF_OF[d],
                        OFF_OF[d],
                        h_child,
                        c_child,
                        hbuf[par],
                        cbuf[par],
                        root_sink=gin if d == SUB_DEPTH else None,
                    )

            # ---- allgather the 8 subtree roots ----
            nc.gpsimd.collective_compute(
                "AllGather",
                mybir.AluOpType.bypass,
                replica_groups=[list(range(NCORES))],
                ins=[gin.opt()],
                outs=[gout.opt()],
            )
            # load gathered roots feature-major: (p, ch, rank); h loads via a
            # casting DMA (gpsimd) straight to bf16 -- no SBUF round trip
            g_ap = gout[:, :].rearrange("(r c p) t -> c p r t", p=128, c=2)
            c_top = ppool.tile([128, 2, 8], f32, tag="ctop")
            h_top = ppool.tile([128, 2, 8], bf16, tag="htop")
            for ch in range(2):
                nc.sync.dma_start(c_top[:, ch, :], g_ap[ch, :, :, 0])
                nc.gpsimd.dma_start(h_top[:, ch, :], g_ap[ch, :, :, 1])

            # ---- top levels 2..0 (computed redundantly on every core) ----
            prev_h, prev_c = h_top, c_top
            for d in (2, 1, 0):
                S = 2**d
                node0 = S - 1
                col0 = TOP_COL0 + node0
                h_d = ppool.tile([128, 2, S], bf16, tag=f"ht{d}", name=f"ht{d}")
                c_d = ppool.tile([128, 2, S], f32, tag=f"ct{d}", name=f"ct{d}")
                do_small_level(
                    S,
                    col0,
                    col0,
                    prev_h[:, :, : 2 * S],
                    prev_c[:, :, : 2 * S],
                    h_d,
                    c_d,
                )
                prev_h, prev_c = h_d, c_d

    nc.compile()
    return nc


def _get_program():
    if "nc" not in _CACHE:
        _CACHE["nc"] = _build_program()
    return _CACHE["nc"]


def _preprocess(inputs, W_ioux, b_ioux, W_iouh, b_iouh, W_fx, b_fx, W_fh, b_fh):
    """Build per-core input maps (numpy only)."""
    bf = ml_dtypes.bfloat16
    wx_cat = np.concatenate([W_ioux, W_fx], axis=0)  # [1024, 300]
    b_cat = np.concatenate([b_ioux + b_iouh, b_fx + b_fh], axis=0)  # [1024]
    # K padded to 384 with zero weight rows (keeps FWL on for every matmul)
    wx_full = np.zeros((K_PAD, 4 * MEM), np.float32)
    wx_full[:IN_DIM] = wx_cat.T
    wx_full[IN_DIM] = b_cat
    wx_np = np.ascontiguousarray(wx_full).astype(bf)
    wh_np = np.ascontiguousarray(W_iouh.T).astype(bf)  # [256, 768]
    wf_np = np.ascontiguousarray(W_fh.T).astype(bf)  # [256, 256]

    xT = np.zeros((K_PAD, N_NODES), np.float32)
    xT[:IN_DIM] = inputs.T
    xT[IN_DIM] = 1.0
    xT = xT.astype(bf)  # [384, 65535]

    in_maps = []
    for j in range(NCORES):
        segs = []
        for d in LEVELS:
            S = S_OF[d]
            g0 = (2**d - 1) + j * S
            segs.append(np.arange(g0, g0 + S))
        segs.append(np.arange(0, 7))
        cols = np.concatenate(segs)
        xcore = np.ascontiguousarray(xT[:, cols])
        # duplicated columns of the small-level x range (per-child f gates)
        xdup = np.ascontiguousarray(np.repeat(xcore[:, XSB_COL0:], 2, axis=1))
        in_maps.append(
            {"xt": xcore, "xdup": xdup, "wx": wx_np, "wh": wh_np, "wf": wf_np}
        )
    return in_maps


def _postprocess(results):
    """Assemble [2, N, 256] from per-core outputs (c from `out` row 0, h from
    the bf16 `outh`, upcast on host)."""
    full = np.empty((2, N_NODES, MEM), np.float32)
    for j in range(NCORES):
        rc = results[j]["out"]  # [2(ch), 128, N_COLS] f32
        rh = results[j]["outh"].astype(np.float32)  # [2(ch), 128, N_COLS]
        r = np.stack([rc, rh])  # [2(c/h), 2, 128, N_COLS]
        for d in LEVELS:
            S = S_OF[d]
            g0 = (2**d - 1) + j * S
            off = OFF_OF[d]
            blk = r[:, :, :, off : off + S]  # [2,2,128,S]
            full[:, g0 : g0 + S, :] = blk.transpose(0, 3, 1, 2).reshape(2, S, MEM)
        if j == 0:
            r0 = r[:, :, :, TOP_COL0 : TOP_COL0 + 7]
            full[:, 0:7, :] = r0.transpose(0, 3, 1, 2).reshape(2, 7, MEM)
    return full


def kernel(**inputs):
    from concourse.bass_utils import run_bass_kernel_spmd

    nc = _get_program()
    inputs = {k: np.asarray(v) for k, v in inputs.items()}
    in_maps = _preprocess(**inputs)
    res = run_bass_kernel_spmd(nc, in_maps, core_ids=list(range(NCORES)))
    _CACHE["last_result"] = res
    return _postprocess(res.results)

